# revision 5
# baseline (speedup 1.0000x reference)
"""Conv-RFF Trainium2 kernel: grouped 3x3/s2 conv (10 MC groups sharing input)
+ cos/sin random-feature epilogue, data-parallel over batch on 8 NeuronCores.

Strategy per core (one batch image each):
  - Build a row-gathered tensor Kb[96, 64, 128] in SBUF via 6 DMAs:
      partitions 0..47  = "Gs" = x rows (2*oy+ki-1) shifted right 1 col (tap kj=0)
      partitions 48..95 = "G"  = x rows (2*oy+ki-1) unshifted          (taps kj=1,2)
  - Conv becomes 2 accumulating matmuls per 512-pixel block:
      mm1: K=96  lhsT=W[kj0|kj1] rhs=Kb[:, oy, 0:128:2]   (even cols)
      mm2: K=48  lhsT=W[kj2]     rhs=Kb[48:96, oy, 1:128:2] (odd cols)
  - Weights are pre-scaled by 1/(2*pi) so PSUM holds u = ph/(2*pi) ("turns").
  - Epilogue: round-to-nearest via the fp32 magic constant (1.5*2^23), then
    ScalarE Sin on the reduced argument (|arg| <= pi, where HW Sin is exact),
    with the quarter-turn shift for cos folded into a second reduction.
"""

import sys

sys.path.insert(0, "/opt/trn_rl_repo")

import numpy as np

MC, IN_C, OUT_C, KK = 10, 16, 32, 3
D = IN_C * KK * KK  # 144
B, H, W = 8, 128, 128
HO = 64  # output spatial
N_RF = OUT_C * HO * HO
PI = float(np.pi)
MAGIC = float(1.5 * 2**23)
N_CORES = 8


def _build_program(c_scale: float):
    import concourse.bacc as bacc
    import concourse.mybir as mybir
    from concourse.tile import TileContext

    f32 = mybir.dt.float32
    AF = mybir.ActivationFunctionType
    ALU = mybir.AluOpType

    nc = bacc.Bacc("TRN2", target_bir_lowering=False, num_devices=N_CORES)
    xb = nc.dram_tensor("xb", [IN_C, H, W], f32, kind="ExternalInput")
    wp = nc.dram_tensor("wp", [96, 320], f32, kind="ExternalInput")
    w2 = nc.dram_tensor("w2", [48, 320], f32, kind="ExternalInput")
    out = nc.dram_tensor("out", [MC, 2, OUT_C, 8, 512], f32, kind="ExternalOutput")

    with TileContext(nc) as tc:
        with (
            tc.tile_pool(name="kbp", bufs=1) as kbp,
            tc.tile_pool(name="wpl", bufs=1) as wpl,
            tc.tile_pool(name="cst", bufs=1) as cst,
            tc.tile_pool(name="sb", bufs=3) as sb,
            tc.tile_pool(name="ob", bufs=4) as ob,
            tc.tile_pool(name="pp", bufs=6, space="PSUM") as pp,
        ):
            kb = kbp.tile([96, 64, 128], f32)
            wpt = wpl.tile([96, 320], f32)
            w2t = wpl.tile([48, 320], f32)
            hpi = cst.tile([128, 1], f32)
            nc.gpsimd.memset(hpi[:], PI / 2)
            nc.sync.dma_start(wpt[:], wp[:])
            nc.sync.dma_start(w2t[:], w2[:])

            # zero padding slivers: shifted col 0, and oy=0 row for ki=0 taps
            nc.gpsimd.memset(kb[32:64, :, 0:1], 0.0)
            nc.gpsimd.memset(kb[64:96, :, 0:1], 0.0)
            nc.gpsimd.memset(kb[0:16, 0:1, :], 0.0)
            nc.gpsimd.memset(kb[32:64, 0:1, :], 0.0)

            # G (unshifted) at partitions 0..47; row sets per ki
            nc.sync.dma_start(kb[0:16, 1:64, :], xb[:, 1:127:2, :])    # ki=0
            nc.sync.dma_start(kb[16:32, :, :], xb[:, 0:128:2, :])      # ki=1
            nc.sync.dma_start(kb[32:48, :, :], xb[:, 1:128:2, :])      # ki=2
            # Gs (shifted right 1 col) at partitions 48..95
            nc.sync.dma_start(kb[48:64, 1:64, 1:128], xb[:, 1:127:2, 0:127])
            nc.sync.dma_start(kb[64:80, :, 1:128], xb[:, 0:128:2, 0:127])
            nc.sync.dma_start(kb[80:96, :, 1:128], xb[:, 1:128:2, 0:127])

            chunks = [(0, 128), (128, 128), (256, 64)]
            for c0, mcs in chunks:
                nmc = mcs // OUT_C
                mc0 = c0 // OUT_C
                for j in range(8):
                    ps = pp.tile([mcs, 512], f32, tag="ps")
                    nc.tensor.matmul(
                        ps[:],
                        wpt[:, c0 : c0 + mcs],
                        kb[:, j * 8 : (j + 1) * 8, 0:128:2],
                        start=True,
                        stop=False,
                    )
                    nc.tensor.matmul(
                        ps[:],
                        w2t[:, c0 : c0 + mcs],
                        kb[0:48, j * 8 : (j + 1) * 8, 1:128:2],
                        start=False,
                        stop=True,
                    )
                    uc = sb.tile([mcs, 512], f32, tag="uc")
                    nc.scalar.activation(uc[:], ps[:], AF.Identity)
                    t1s = sb.tile([mcs, 512], f32, tag="t1s")
                    nc.gpsimd.tensor_scalar(t1s[:], uc[:], MAGIC, None, op0=ALU.add)
                    t1c = sb.tile([mcs, 512], f32, tag="t1c")
                    nc.gpsimd.tensor_scalar(
                        t1c[:], uc[:], 0.25, MAGIC, op0=ALU.add, op1=ALU.add
                    )
                    gs = sb.tile([mcs, 512], f32, tag="gs")
                    nc.vector.scalar_tensor_tensor(
                        gs[:], t1s[:], MAGIC, uc[:], op0=ALU.subtract, op1=ALU.subtract
                    )
                    gc = sb.tile([mcs, 512], f32, tag="gc")
                    nc.vector.scalar_tensor_tensor(
                        gc[:], t1c[:], MAGIC, uc[:], op0=ALU.subtract, op1=ALU.subtract
                    )
                    sn = sb.tile([mcs, 512], f32, tag="sn")
                    nc.scalar.activation(sn[:], gs[:], AF.Sin, scale=-2 * PI)
                    cs = sb.tile([mcs, 512], f32, tag="cs")
                    nc.scalar.activation(
                        cs[:], gc[:], AF.Sin, bias=hpi[:mcs, :], scale=-2 * PI
                    )
                    msn = ob.tile([mcs, 512], f32, tag="msn")
                    nc.vector.tensor_scalar_mul(msn[:], sn[:], c_scale)
                    mcs_t = ob.tile([mcs, 512], f32, tag="mcs")
                    nc.gpsimd.tensor_scalar_mul(mcs_t[:], cs[:], c_scale)
                    for m in range(nmc):
                        nc.sync.dma_start(
                            out[mc0 + m, 0, :, j, :], mcs_t[m * 32 : (m + 1) * 32, :]
                        )
                        nc.sync.dma_start(
                            out[mc0 + m, 1, :, j, :], msn[m * 32 : (m + 1) * 32, :]
                        )
    nc.compile()
    return nc


def _prep_weights(theta_logsigma, Omega_mean, Omega_logsigma, Omega_eps):
    om = Omega_eps.astype(np.float64) * np.exp(
        Omega_logsigma.astype(np.float64) * 0.5
    ) + Omega_mean.astype(np.float64)
    wd = om.transpose(1, 0, 2).reshape(D, MC * OUT_C)  # [d, mc*32+oc]
    wt = (wd / (2 * np.pi)).reshape(KK, KK, IN_C, MC * OUT_C)
    kj0 = wt[:, 0].reshape(48, MC * OUT_C)
    kj1 = wt[:, 1].reshape(48, MC * OUT_C)
    kj2 = wt[:, 2].reshape(48, MC * OUT_C)
    wpair = np.ascontiguousarray(
        np.concatenate([kj1, kj0], axis=0), dtype=np.float32
    )
    wk2 = np.ascontiguousarray(kj2, dtype=np.float32)
    c_scale = float(np.exp(0.5 * float(theta_logsigma[0])) / np.sqrt(N_RF))
    return wpair, wk2, c_scale


def kernel(x, theta_logsigma, Omega_mean, Omega_logsigma, Omega_eps):
    from concourse.bass_utils import run_bass_kernel_spmd

    wpair, wk2, c_scale = _prep_weights(
        theta_logsigma, Omega_mean, Omega_logsigma, Omega_eps
    )
    nc = _build_program(c_scale)
    x = np.ascontiguousarray(x, dtype=np.float32)
    in_maps = [
        {"xb": x[i], "wp": wpair, "w2": wk2} for i in range(N_CORES)
    ]
    res = run_bass_kernel_spmd(nc, in_maps, list(range(N_CORES)))
    outs = [res.results[i]["out"].reshape(2 * MC * OUT_C, HO, HO) for i in range(N_CORES)]
    return np.stack(outs, axis=0)


if __name__ == "__main__":
    rng = np.random.default_rng(0)
    ins = {
        "x": rng.standard_normal((B, IN_C, H, W), dtype=np.float32),
        "theta_logsigma": np.zeros((1,), np.float32),
        "Omega_mean": np.zeros((D, OUT_C), np.float32),
        "Omega_logsigma": np.full((D, OUT_C), -np.log(float(D)), np.float32),
        "Omega_eps": rng.standard_normal((MC, D, OUT_C), dtype=np.float32),
    }
    out = kernel(**ins)
    print(out.shape, out.dtype)


# revision 7
# speedup vs baseline: 1.0795x; 1.0795x over previous
"""Conv-RFF Trainium2 kernel: grouped 3x3/s2 conv (10 MC groups sharing input)
+ cos/sin random-feature epilogue, data-parallel over batch on 8 NeuronCores.

Strategy per core (one batch image each):
  - Build a row-gathered tensor Kb[96, 64, 128] in SBUF via 6 DMAs:
      partitions 0..47  = "Gs" = x rows (2*oy+ki-1) shifted right 1 col (tap kj=0)
      partitions 48..95 = "G"  = x rows (2*oy+ki-1) unshifted          (taps kj=1,2)
  - Conv becomes 2 accumulating matmuls per 512-pixel block:
      mm1: K=96  lhsT=W[kj0|kj1] rhs=Kb[:, oy, 0:128:2]   (even cols)
      mm2: K=48  lhsT=W[kj2]     rhs=Kb[48:96, oy, 1:128:2] (odd cols)
  - Weights are pre-scaled by 1/(2*pi) so PSUM holds u = ph/(2*pi) ("turns").
  - Epilogue: round-to-nearest via the fp32 magic constant (1.5*2^23), then
    ScalarE Sin on the reduced argument (|arg| <= pi, where HW Sin is exact),
    with the quarter-turn shift for cos folded into a second reduction.
"""

import sys

sys.path.insert(0, "/opt/trn_rl_repo")

import numpy as np

MC, IN_C, OUT_C, KK = 10, 16, 32, 3
D = IN_C * KK * KK  # 144
B, H, W = 8, 128, 128
HO = 64  # output spatial
N_RF = OUT_C * HO * HO
PI = float(np.pi)
MAGIC = float(1.5 * 2**23)
N_CORES = 8


def _build_program(c_scale: float, JW: int = 2, SB_BUFS: int = 3, PS_BUFS: int = 3, OB_BUFS: int = 4):
    import concourse.bacc as bacc
    import concourse.mybir as mybir
    from concourse.tile import TileContext

    f32 = mybir.dt.float32
    AF = mybir.ActivationFunctionType
    ALU = mybir.AluOpType

    nc = bacc.Bacc("TRN2", target_bir_lowering=False, num_devices=N_CORES)
    xb = nc.dram_tensor("xb", [IN_C, H, W], f32, kind="ExternalInput")
    wp = nc.dram_tensor("wp", [96, 320], f32, kind="ExternalInput")
    w2 = nc.dram_tensor("w2", [48, 320], f32, kind="ExternalInput")
    out = nc.dram_tensor("out", [MC, 2, OUT_C, 8, 512], f32, kind="ExternalOutput")

    with TileContext(nc) as tc:
        with (
            tc.tile_pool(name="kbp", bufs=1) as kbp,
            tc.tile_pool(name="wpl", bufs=1) as wpl,
            tc.tile_pool(name="cst", bufs=1) as cst,
            tc.tile_pool(name="sb", bufs=SB_BUFS) as sb,
            tc.tile_pool(name="ob", bufs=OB_BUFS) as ob,
            tc.tile_pool(name="pp", bufs=PS_BUFS, space="PSUM") as pp,
        ):
            kb = kbp.tile([96, 64, 128], f32)
            wpt = wpl.tile([96, 320], f32)
            w2t = wpl.tile([48, 320], f32)
            hpi = cst.tile([128, 1], f32)
            nc.gpsimd.memset(hpi[:], PI / 2)
            nc.sync.dma_start(wpt[:], wp[:])
            nc.sync.dma_start(w2t[:], w2[:])

            # zero padding slivers: shifted col 0, and oy=0 row for ki=0 taps
            nc.gpsimd.memset(kb[32:64, :, 0:1], 0.0)
            nc.gpsimd.memset(kb[64:96, :, 0:1], 0.0)
            nc.gpsimd.memset(kb[0:16, 0:1, :], 0.0)
            nc.gpsimd.memset(kb[32:64, 0:1, :], 0.0)

            # G (unshifted) at partitions 0..47; row sets per ki
            nc.sync.dma_start(kb[0:16, 1:64, :], xb[:, 1:127:2, :])    # ki=0
            nc.sync.dma_start(kb[16:32, :, :], xb[:, 0:128:2, :])      # ki=1
            nc.sync.dma_start(kb[32:48, :, :], xb[:, 1:128:2, :])      # ki=2
            # Gs (shifted right 1 col) at partitions 48..95
            nc.sync.dma_start(kb[48:64, 1:64, 1:128], xb[:, 1:127:2, 0:127])
            nc.sync.dma_start(kb[64:80, :, 1:128], xb[:, 0:128:2, 0:127])
            nc.sync.dma_start(kb[80:96, :, 1:128], xb[:, 1:128:2, 0:127])

            chunks = [(0, 128), (128, 128), (256, 64)]
            for c0, mcs in chunks:
                nmc = mcs // OUT_C
                mc0 = c0 // OUT_C
                for jb in range(8 // JW):
                    ps = pp.tile([mcs, JW, 512], f32, tag="ps")
                    for jj in range(JW):
                        j = jb * JW + jj
                        nc.tensor.matmul(
                            ps[:, jj, :],
                            wpt[:, c0 : c0 + mcs],
                            kb[:, j * 8 : (j + 1) * 8, 0:128:2],
                            start=True,
                            stop=False,
                        )
                        nc.tensor.matmul(
                            ps[:, jj, :],
                            w2t[:, c0 : c0 + mcs],
                            kb[0:48, j * 8 : (j + 1) * 8, 1:128:2],
                            start=False,
                            stop=True,
                        )
                    uc = sb.tile([mcs, JW, 512], f32, tag="uc")
                    nc.scalar.activation(uc[:], ps[:], AF.Identity)
                    t1s = sb.tile([mcs, JW, 512], f32, tag="t1s")
                    nc.gpsimd.tensor_scalar(t1s[:], uc[:], MAGIC, None, op0=ALU.add)
                    t1c = sb.tile([mcs, JW, 512], f32, tag="t1c")
                    nc.gpsimd.tensor_scalar(
                        t1c[:], uc[:], 0.25, MAGIC, op0=ALU.add, op1=ALU.add
                    )
                    gs = sb.tile([mcs, JW, 512], f32, tag="gs")
                    nc.vector.scalar_tensor_tensor(
                        gs[:], t1s[:], MAGIC, uc[:], op0=ALU.subtract, op1=ALU.subtract
                    )
                    gc = sb.tile([mcs, JW, 512], f32, tag="gc")
                    nc.vector.scalar_tensor_tensor(
                        gc[:], t1c[:], MAGIC, uc[:], op0=ALU.subtract, op1=ALU.subtract
                    )
                    sn = sb.tile([mcs, JW, 512], f32, tag="sn")
                    nc.scalar.activation(sn[:], gs[:], AF.Sin, scale=-2 * PI)
                    cs = sb.tile([mcs, JW, 512], f32, tag="cs")
                    nc.scalar.activation(
                        cs[:], gc[:], AF.Sin, bias=hpi[:mcs, :], scale=-2 * PI
                    )
                    msn = ob.tile([mcs, JW, 512], f32, tag="msn")
                    nc.vector.tensor_scalar_mul(msn[:], sn[:], c_scale)
                    mcs_t = ob.tile([mcs, JW, 512], f32, tag="mcs")
                    nc.gpsimd.tensor_scalar_mul(mcs_t[:], cs[:], c_scale)
                    for m in range(nmc):
                        nc.sync.dma_start(
                            out[mc0 + m, 0, :, jb * JW : (jb + 1) * JW, :],
                            mcs_t[m * 32 : (m + 1) * 32, :, :],
                        )
                        nc.sync.dma_start(
                            out[mc0 + m, 1, :, jb * JW : (jb + 1) * JW, :],
                            msn[m * 32 : (m + 1) * 32, :, :],
                        )
    nc.compile()
    return nc


def _prep_weights(theta_logsigma, Omega_mean, Omega_logsigma, Omega_eps):
    om = Omega_eps.astype(np.float64) * np.exp(
        Omega_logsigma.astype(np.float64) * 0.5
    ) + Omega_mean.astype(np.float64)
    wd = om.transpose(1, 0, 2).reshape(D, MC * OUT_C)  # [d, mc*32+oc]
    wt = (wd / (2 * np.pi)).reshape(KK, KK, IN_C, MC * OUT_C)
    kj0 = wt[:, 0].reshape(48, MC * OUT_C)
    kj1 = wt[:, 1].reshape(48, MC * OUT_C)
    kj2 = wt[:, 2].reshape(48, MC * OUT_C)
    wpair = np.ascontiguousarray(
        np.concatenate([kj1, kj0], axis=0), dtype=np.float32
    )
    wk2 = np.ascontiguousarray(kj2, dtype=np.float32)
    c_scale = float(np.exp(0.5 * float(theta_logsigma[0])) / np.sqrt(N_RF))
    return wpair, wk2, c_scale


def kernel(x, theta_logsigma, Omega_mean, Omega_logsigma, Omega_eps):
    from concourse.bass_utils import run_bass_kernel_spmd

    wpair, wk2, c_scale = _prep_weights(
        theta_logsigma, Omega_mean, Omega_logsigma, Omega_eps
    )
    nc = _build_program(c_scale)
    x = np.ascontiguousarray(x, dtype=np.float32)
    in_maps = [
        {"xb": x[i], "wp": wpair, "w2": wk2} for i in range(N_CORES)
    ]
    res = run_bass_kernel_spmd(nc, in_maps, list(range(N_CORES)))
    outs = [res.results[i]["out"].reshape(2 * MC * OUT_C, HO, HO) for i in range(N_CORES)]
    return np.stack(outs, axis=0)


if __name__ == "__main__":
    rng = np.random.default_rng(0)
    ins = {
        "x": rng.standard_normal((B, IN_C, H, W), dtype=np.float32),
        "theta_logsigma": np.zeros((1,), np.float32),
        "Omega_mean": np.zeros((D, OUT_C), np.float32),
        "Omega_logsigma": np.full((D, OUT_C), -np.log(float(D)), np.float32),
        "Omega_eps": rng.standard_normal((MC, D, OUT_C), dtype=np.float32),
    }
    out = kernel(**ins)
    print(out.shape, out.dtype)
